# revision 10
# baseline (speedup 1.0000x reference)

# BiMamba2 block on 8 NeuronCores (TRN2, Bass/Tile).
#
# Sharding: 2 batches x 4 head-groups (8 heads / 512 channels each core).
# Each core computes, for its (batch b, head-group g) and BOTH directions:
#   in_proj slice -> depthwise conv (causal fwd / anticausal rev, both in
#   forward time order) -> silu -> chunked SSD (Q=128 chunks, quarter-split
#   re-centered exp factorization of the decay kernel) -> gate with silu(z)
#   -> partial out_proj (row-parallel over channels) + partial sum-of-squares
#   for the gated RMSNorm.
# Host combines: out = rsqrt(mean(ssq)+eps) * sum_g(partial) per direction,
# summed over directions.  The per-row RMS scale commutes with W_out, which
# is what makes row-parallel sharding of out_proj exact.
#
# The reverse direction is computed as an anticausal scan in forward time
# order (mirror image of the causal one), so all buffers/layouts are shared
# with the forward direction and no sequence reversal happens anywhere.

import sys
import numpy as np

for _p in ("/opt/trn_rl_repo", "/root/.axon_site/_ro/trn_rl_repo"):
    if _p not in sys.path:
        sys.path.insert(0, _p)

import ml_dtypes

BF16 = ml_dtypes.bfloat16

D_MODEL = 1024
D_INNER = 2048
NHEADS = 32
HEADDIM = 64
T = 2048
Q = 128                                    # chunk length
NCH = T // Q                               # 16 chunks
NQ = 32                                    # quarter size
CH = 512                                   # channels per core (8 heads)
HD = 8                                     # heads per core
KT = 8                                     # 1024 / 128 contraction tiles
TC = 4                                     # t-tiles of 512
EPS = 1e-5

XBC_W = T + 6                              # padded conv row length (2054)
NXBCT = 6                                  # xBC channel tiles (512 xs + 128 B + 128 C)


def build_program():
    from concourse import bacc, mybir
    import concourse.tile as tile

    f32 = mybir.dt.float32
    bf16 = mybir.dt.bfloat16
    f16 = mybir.dt.float16
    AF = mybir.ActivationFunctionType
    OP = mybir.AluOpType

    nc = bacc.Bacc("TRN2", target_bir_lowering=False, debug=False, num_devices=8)

    # ---------------- DRAM I/O ----------------
    f32r_t = __import__("concourse.mybir", fromlist=["dt"]).dt.float32r
    xT = nc.dram_tensor("xT", [D_MODEL, T], f32r_t, kind="ExternalInput").ap()
    wz = nc.dram_tensor("wz", [D_MODEL, CH], f32r_t, kind="ExternalInput").ap()
    wxbc = nc.dram_tensor("wxbc", [D_MODEL, 776], f32r_t, kind="ExternalInput").ap()
    wout = nc.dram_tensor("wout", [CH, 2048], bf16, kind="ExternalInput").ap()
    convw = nc.dram_tensor("convw", [128, 48], f32, kind="ExternalInput").ap()
    convb = nc.dram_tensor("convb", [128, 12], f32, kind="ExternalInput").ap()
    dtbias = nc.dram_tensor("dtbias", [8, 2], f32, kind="ExternalInput").ap()
    arow = nc.dram_tensor("arow", [1, 256], f32, kind="ExternalInput").ap()
    cst_bf = nc.dram_tensor("cst_bf", [128, 384], bf16, kind="ExternalInput").ap()
    cst_f = nc.dram_tensor("cst_f", [128, 384], f32, kind="ExternalInput").ap()
    qind = nc.dram_tensor("qind", [4, 128], f32, kind="ExternalInput").ap()
    onesrow = nc.dram_tensor("onesrow", [1, 128], f32, kind="ExternalInput").ap()
    zrow = nc.dram_tensor("zrow", [1, 128], f32, kind="ExternalInput").ap()
    selm = nc.dram_tensor("selm", [128, 1280], f32, kind="ExternalInput").ap()

    outT_f = nc.dram_tensor("outT_f", [D_MODEL, T], f16, kind="ExternalOutput").ap()
    outT_r = nc.dram_tensor("outT_r", [D_MODEL, T], f16, kind="ExternalOutput").ap()
    ssq_o = nc.dram_tensor("ssq", [128, 32], f32, kind="ExternalOutput").ap()

    with tile.TileContext(nc) as tc:
        with (
            tc.tile_pool(name="consts", bufs=1) as cpool,
            tc.tile_pool(name="wout_p", bufs=1) as wopool,
            tc.tile_pool(name="recA", bufs=1) as recA,      # wxbc -> gt_f
            tc.tile_pool(name="recB", bufs=1) as recB,      # wz   -> gt_r
            tc.tile_pool(name="xtr", bufs=2) as xtr,        # xt stream -> cxs_f/cxs_r
            tc.tile_pool(name="bigbuf", bufs=1) as bpool,
            tc.tile_pool(name="dtprep", bufs=1) as dpool,
            tc.tile_pool(name="dtscr", bufs=2) as dscr,
            tc.tile_pool(name="xst", bufs=4) as xstpool,
            tc.tile_pool(name="ssdtmp", bufs=2) as stpool,
            tc.tile_pool(name="ytmp", bufs=2) as ypool,
            tc.tile_pool(name="gtile", bufs=5) as gpool,
            tc.tile_pool(name="outstg", bufs=2) as opool,
            tc.tile_pool(name="psA", bufs=4, space="PSUM") as psA,   # 512-wide f32
            tc.tile_pool(name="psB", bufs=2, space="PSUM") as psB,   # 128-wide f32
        ):
            # ---------- consts ----------
            idbf = cpool.tile([128, 128], bf16)
            maskbd = cpool.tile([128, 256], bf16)
            nc.sync.dma_start(idbf[:], cst_bf[:, 0:128])
            nc.sync.dma_start(maskbd[:], cst_bf[:, 128:384])
            idf = cpool.tile([128, 128], f32)
            cumW = cpool.tile([128, 256], f32)
            nc.sync.dma_start(idf[:], cst_f[:, 0:128])
            nc.sync.dma_start(cumW[:], cst_f[:, 128:384])
            qind_sb = cpool.tile([4, 128], f32)
            nc.sync.dma_start(qind_sb[:], qind[:])
            ones_sb = cpool.tile([1, 128], f32)
            nc.sync.dma_start(ones_sb[:], onesrow[:])
            zrow_sb = cpool.tile([1, 128], f32)
            nc.sync.dma_start(zrow_sb[:], zrow[:])
            convw_sb = cpool.tile([128, 48], f32)
            convb_sb = cpool.tile([128, 12], f32)
            dtb_sb = cpool.tile([8, 2], f32)
            arow_sb = cpool.tile([1, 256], f32)
            selm_sb = cpool.tile([128, 1280], f32)
            nc.sync.dma_start(selm_sb[:], selm[:])
            nc.sync.dma_start(convw_sb[:], convw[:])
            nc.sync.dma_start(convb_sb[:], convb[:])
            nc.sync.dma_start(dtb_sb[:], dtbias[:])
            nc.sync.dma_start(arow_sb[:], arow[:])

            wout_sb = wopool.tile([128, 4 * 2048], bf16)
            nc.sync.dma_start(wout_sb[:].rearrange("p (k o) -> p k o", k=4), wout.rearrange("(k p) o -> p k o", p=128))
            # recycled: weights now, gt buffers later
            wxbc_sb = recA.tile([128, KT * 776], mybir.dt.float32r, tag="rA")
            nc.sync.dma_start(wxbc_sb[:].rearrange("p (k c) -> p k c", k=KT), wxbc.rearrange("(k p) c -> p k c", p=128))
            wz_sb = recB.tile([128, KT * CH], mybir.dt.float32r, tag="rB")
            nc.sync.dma_start(wz_sb[:].rearrange("p (k c) -> p k c", k=KT), wz.rearrange("(k p) c -> p k c", p=128))

            def f32r(ap):
                return ap.bitcast(mybir.dt.float32r)

            # ---------- big persistent buffers ----------
            xbc = bpool.tile([128, NXBCT * XBC_W], bf16)     # in_proj xBC (ch x t), padded
            zsil = bpool.tile([128, NCH * CH], bf16)         # silu(z) (t x ch), chunk-major
            cB = [bpool.tile([128, T], bf16, name=f"cB{i}") for i in range(2)]
            cC = [bpool.tile([128, T], bf16, name=f"cC{i}") for i in range(2)]
            dtraw = bpool.tile([8, 2 * T], f32)
            carry = [bpool.tile([128, CH], bf16, name=f"carry{i}") for i in range(2)]
            ssq_sb = bpool.tile([128, 32], f32)

            for ct in range(NXBCT):
                nc.vector.memset(xbc[:, ct * XBC_W: ct * XBC_W + 3], 0.0)
                nc.vector.memset(xbc[:, ct * XBC_W + 3 + T: (ct + 1) * XBC_W], 0.0)

            # ---------- Phase B: in_proj ----------
            for tcv in range(TC):
                xt = xtr.tile([128, KT * 512], mybir.dt.float32r, tag="xtr")
                nc.sync.dma_start(
                    xt[:].rearrange("p (k t) -> p k t", k=KT),
                    xT[:, tcv * 512:(tcv + 1) * 512].rearrange("(k p) t -> p k t", p=128),
                )
                for ct in range(NXBCT):
                    ps = psA.tile([128, 512], f32, tag="psA")
                    for k in range(KT):
                        nc.tensor.matmul(
                            ps[:],
                            wxbc_sb[:, k * 776 + ct * 128: k * 776 + (ct + 1) * 128],
                            xt[:, k * 512:(k + 1) * 512],
                            start=(k == 0), stop=(k == KT - 1),
                        )
                    nc.vector.tensor_copy(
                        xbc[:, ct * XBC_W + 3 + tcv * 512: ct * XBC_W + 3 + (tcv + 1) * 512],
                        ps[:],
                    )
                psd = psB.tile([8, 512], f32, tag="psB")
                for k in range(KT):
                    nc.tensor.matmul(
                        psd[:],
                        wxbc_sb[:, k * 776 + 768: k * 776 + 776],
                        xt[:, k * 512:(k + 1) * 512],
                        start=(k == 0), stop=(k == KT - 1),
                    )
                for d in range(2):
                    nc.vector.tensor_scalar(
                        dtraw[:, d * T + tcv * 512: d * T + (tcv + 1) * 512],
                        psd[:], dtb_sb[:, d:d + 1], None, OP.add,
                    )
                for sub in range(4):
                    cg = tcv * 4 + sub
                    psz = psA.tile([128, 512], f32, tag="psA")
                    for k in range(KT):
                        nc.tensor.matmul(
                            psz[:],
                            xt[:, k * 512 + sub * 128: k * 512 + (sub + 1) * 128],
                            wz_sb[:, k * CH:(k + 1) * CH],
                            start=(k == 0), stop=(k == KT - 1),
                        )
                    nc.scalar.activation(zsil[:, cg * CH:(cg + 1) * CH], psz[:], AF.Silu)

            # conv xs outputs: one tile per dir (4 chtiles side by side), recycling
            # the xt slots (in_proj is fully done before conv starts).
            cxs = [xtr.tile([128, 4 * T], bf16, tag="xtr", name=f"cxs{i}") for i in range(2)]

            # ---------- Phase C: conv ----------
            for d in range(2):
                for ct in range(NXBCT):
                    if ct < 4:
                        dst_full = cxs[d][:, ct * T:(ct + 1) * T]
                    elif ct == 4:
                        dst_full = cB[d][:]
                    else:
                        dst_full = cC[d][:]
                    base = ct * XBC_W
                    for tt in range(TC):
                        o = dst_full[:, tt * 512:(tt + 1) * 512]
                        sh0 = base + tt * 512 + (0 if d == 0 else 3)
                        nc.vector.tensor_scalar(
                            o, xbc[:, sh0: sh0 + 512],
                            convw_sb[:, d * 24 + ct * 4: d * 24 + ct * 4 + 1],
                            convb_sb[:, d * 6 + ct: d * 6 + ct + 1],
                            OP.mult, OP.add,
                        )
                        for j in (1, 2, 3):
                            sh = base + tt * 512 + j + (0 if d == 0 else 3)
                            nc.vector.scalar_tensor_tensor(
                                o, xbc[:, sh: sh + 512],
                                convw_sb[:, d * 24 + ct * 4 + j: d * 24 + ct * 4 + j + 1],
                                o, OP.mult, OP.add,
                            )
                nc.scalar.activation(cB[d][:], cB[d][:], AF.Silu)
                nc.scalar.activation(cC[d][:], cC[d][:], AF.Silu)

            # ---------- Phase D: dt prep ----------
            dtc, urel, uchk, dvt, dst_, dbt, owq = [], [], [], [], [], [], []
            for d in range(2):
                dtcd = dpool.tile([128, 128], f32, tag=f"dtc{d}")
                for grp in range(4):
                    pst = psB.tile([128, 32], f32, tag="psB")
                    for j in range(4):
                        c = grp * 4 + j
                        nc.tensor.transpose(
                            pst[:, 8 * j: 8 * (j + 1)],
                            dtraw[:8, d * T + c * Q: d * T + (c + 1) * Q],
                            idf[:8, :8],
                        )
                    spt = dscr.tile([128, 32], f32, tag="dscr32")
                    nc.scalar.activation(spt[:], pst[:], AF.Exp)
                    nc.scalar.activation(
                        dtcd[:, 32 * grp: 32 * (grp + 1)], spt[:], AF.Ln, bias=1.0
                    )
                dtc.append(dtcd)
                psa = psB.tile([128, 128], f32, tag="psB")
                nc.tensor.matmul(psa[:], ones_sb[:], arow_sb[:, d * 128:(d + 1) * 128],
                                 start=True, stop=True)
                ab = dscr.tile([128, 128], f32, tag="dscr")
                nc.vector.tensor_tensor(ab[:], dtcd[:], psa[:], OP.mult)
                psc = psB.tile([128, 128], f32, tag="psB")
                nc.tensor.matmul(psc[:], cumW[:, d * 128:(d + 1) * 128], ab[:],
                                 start=True, stop=True)
                cums = dpool.tile([128, 128], f32, tag=f"cums{d}")
                nc.vector.tensor_copy(cums[:], psc[:])
                psr = psB.tile([128, 128], f32, tag="psB")
                nc.tensor.matmul(psr[:], selm_sb[:, d * 640: d * 640 + 128], cums[:],
                                 start=True, stop=True)
                crel = dscr.tile([128, 128], f32, tag="dscr")
                nc.vector.tensor_tensor(crel[:], cums[:], psr[:], OP.subtract)
                ur = dpool.tile([128, 128], f32, tag=f"ur{d}")
                nc.scalar.activation(ur[:], crel[:], AF.Exp)
                uc = dpool.tile([128, 128], f32, tag=f"uc{d}")
                nc.scalar.activation(uc[:], cums[:], AF.Exp)
                env = dscr.tile([128, 128], f32, tag="dscr")
                nc.scalar.activation(env[:], crel[:], AF.Exp, scale=-1.0)
                dv = dpool.tile([128, 128], f32, tag=f"dv{d}")
                nc.vector.tensor_tensor(dv[:], dtcd[:], env[:], OP.mult)
                psT = psB.tile([128, 128], f32, tag="psB")
                nc.tensor.matmul(psT[:], selm_sb[:, d * 640 + 128: d * 640 + 256], cums[:],
                                 start=True, stop=True)
                tdif = dscr.tile([128, 128], f32, tag="dscr")
                nc.vector.tensor_tensor(tdif[:], psT[:], cums[:], OP.subtract)
                dse = dscr.tile([128, 128], f32, tag="dscr")
                nc.scalar.activation(dse[:], tdif[:], AF.Exp)
                dsv = dpool.tile([128, 128], f32, tag=f"dsv{d}")
                nc.vector.tensor_tensor(dsv[:], dtcd[:], dse[:], OP.mult)
                dbv = dpool.tile([128, 128], f32, tag=f"dbv{d}")
                nc.scalar.activation(dbv[:], psT[:], AF.Exp)
                owd = {}
                qlist = (1, 2, 3) if d == 0 else (0, 1, 2)
                for qn, qi in enumerate(qlist):
                    psq = psB.tile([128, 128], f32, tag="psB")
                    nc.tensor.matmul(psq[:], selm_sb[:, d * 640 + (2 + qn) * 128:
                                                     d * 640 + (3 + qn) * 128], cums[:],
                                     start=True, stop=True)
                    tq = dscr.tile([128, 128], f32, tag="dscr")
                    nc.vector.tensor_tensor(tq[:], psq[:], cums[:], OP.subtract)
                    eq = dscr.tile([128, 128], f32, tag="dscr")
                    nc.scalar.activation(eq[:], tq[:], AF.Exp)
                    ow = dpool.tile([128, 128], f32, tag=f"ow{d}_{qi}")
                    nc.vector.tensor_tensor(ow[:], dtcd[:], eq[:], OP.mult)
                    owd[qi] = ow
                urel.append(ur); uchk.append(uc); dvt.append(dv)
                dst_.append(dsv); dbt.append(dbv); owq.append(owd)

            # gt buffers (recycled over the in_proj weight slots)
            gt_f = recA.tile([128, NCH * 512], bf16, tag="rA")
            gt_r = recB.tile([128, NCH * 512], bf16, tag="rB")
            gt_bufs = (gt_f, gt_r)

            # ---------- Phase E/F: per-direction SSD + interleaved out_proj ----------
            def bc8(tile128, c, p0=0, pn=128):
                return (tile128[p0:p0 + pn, 8 * c: 8 * (c + 1)]
                        .unsqueeze(2).broadcast_to([pn, 8, 64]))

            outT = (outT_f, outT_r)
            g_keep = {}

            def emit_outproj(d, tcv):
                for ot in range(8):
                    pso = psA.tile([128, 512], f32, tag="psA")
                    for kt in range(4):
                        nc.tensor.matmul(
                            pso[:],
                            wout_sb[:, kt * 2048 + d * 1024 + 128 * ot:
                                    kt * 2048 + d * 1024 + 128 * (ot + 1)],
                            gt_bufs[d][:, (4 * tcv + kt) * 512: (4 * tcv + kt + 1) * 512],
                            start=(kt == 0), stop=(kt == 3),
                        )
                    stg = opool.tile([128, 512], f16, tag="stg")
                    if (ot % 2) == 0:
                        nc.vector.tensor_copy(stg[:], pso[:])
                    else:
                        nc.scalar.copy(stg[:], pso[:])
                    nc.sync.dma_start(
                        outT[d][128 * ot: 128 * (ot + 1), 512 * tcv: 512 * (tcv + 1)],
                        stg[:],
                    )

            for d in range(2):
                chunks = list(range(NCH)) if d == 0 else list(range(NCH - 1, -1, -1))
                for ci, c in enumerate(chunks):
                    first = ci == 0
                    # xs transpose + silu -> xst (t x ch)
                    psX = psA.tile([128, 512], bf16, tag="psAb", bufs=2)
                    for ct in range(4):
                        nc.tensor.transpose(
                            psX[:, 128 * ct: 128 * (ct + 1)],
                            cxs[d][:, ct * T + c * Q: ct * T + (c + 1) * Q], idbf[:],
                        )
                    xst = xstpool.tile([128, 512], bf16, tag="xst")
                    nc.scalar.activation(xst[:], psX[:], AF.Silu)
                    xst3 = xst.rearrange("p (h e) -> p h e", h=8)

                    psG = psB.tile([128, 128], f32, tag="psB")
                    nc.tensor.matmul(psG[:], cB[d][:, c * Q:(c + 1) * Q],
                                     cC[d][:, c * Q:(c + 1) * Q], start=True, stop=True)
                    Gm = stpool.tile([128, 128], bf16, tag="Gm")
                    nc.vector.tensor_tensor(Gm[:], psG[:],
                                            maskbd[:, d * 128:(d + 1) * 128], OP.mult)
                    Graw = stpool.tile([128, 128], bf16, tag="Graw")
                    nc.vector.tensor_copy(Graw[:], psG[:])

                    xv = stpool.tile([128, 512], bf16, tag="xv")
                    nc.vector.tensor_tensor(
                        xv.rearrange("p (h e) -> p h e", h=8), xst3, bc8(dvt[d], c), OP.mult)
                    xs2 = stpool.tile([128, 512], bf16, tag="xs2")
                    nc.vector.tensor_tensor(
                        xs2.rearrange("p (h e) -> p h e", h=8), xst3, bc8(dst_[d], c), OP.mult)

                    psY = psA.tile([128, 512], f32, tag="psA")
                    nc.tensor.matmul(psY[:], Gm[:], xv[:], start=True, stop=False)
                    qlist = (1, 2, 3) if d == 0 else (0, 1, 2)
                    # (dest quarter, stat row base, rows) with PE-legal row bases
                    if d == 0:
                        offmm = [(1, 0, 32), (2, 0, 64), (3, 0, 96)]
                    else:
                        offmm = [(0, 32, 32), (0, 64, 64), (1, 64, 64), (2, 96, 32)]
                    for qi in qlist:
                        xw = stpool.tile([128, 512], bf16, tag="xw", name=f"xw{qi}")
                        nc.vector.tensor_tensor(
                            xw.rearrange("p (h e) -> p h e", h=8),
                            xst3, bc8(owq[d][qi], c), OP.mult)
                        if qi == qlist[0]:
                            xw_by_q = {}
                        xw_by_q[qi] = xw
                    for mi, (qi, s0, sn) in enumerate(offmm):
                        nc.tensor.matmul(
                            psY[32 * qi: 32 * (qi + 1), :],
                            Graw[s0:s0 + sn, 32 * qi: 32 * (qi + 1)],
                            xw_by_q[qi][s0:s0 + sn, :],
                            start=False, stop=(mi == len(offmm) - 1),
                            tile_position=(s0, 32 * qi),
                        )

                    if not first:
                        psO = psA.tile([128, 512], f32, tag="psA")
                        nc.tensor.matmul(psO[:], cC[d][:, c * Q:(c + 1) * Q],
                                         carry[d][:], start=True, stop=True)

                    psBt = psA.tile([128, 128], bf16, tag="psAb", bufs=2)
                    nc.tensor.transpose(psBt[:], cB[d][:, c * Q:(c + 1) * Q], idbf[:])
                    Bt = stpool.tile([128, 128], bf16, tag="Bt")
                    nc.vector.tensor_copy(Bt[:], psBt[:])
                    psS = psA.tile([128, 512], f32, tag="psA")
                    nc.tensor.matmul(psS[:], Bt[:], xs2[:], start=True, stop=True)
                    if first:
                        nc.vector.tensor_copy(carry[d][:], psS[:])
                    else:
                        nc.vector.tensor_tensor(
                            carry[d].rearrange("p (h e) -> p h e", h=8),
                            carry[d].rearrange("p (h e) -> p h e", h=8),
                            bc8(dbt[d], c), OP.mult)
                        nc.vector.tensor_tensor(carry[d][:], carry[d][:], psS[:], OP.add)

                    Ya = ypool.tile([128, 512], bf16, tag="Ya")
                    nc.vector.tensor_tensor(
                        Ya.rearrange("p (h e) -> p h e", h=8),
                        psY.rearrange("p (h e) -> p h e", h=8),
                        bc8(urel[d], c), OP.mult)
                    if not first:
                        Yb = ypool.tile([128, 512], bf16, tag="Yb")
                        nc.vector.tensor_tensor(
                            Yb.rearrange("p (h e) -> p h e", h=8),
                            psO.rearrange("p (h e) -> p h e", h=8),
                            bc8(uchk[d], c), OP.mult)
                        nc.vector.tensor_tensor(Ya[:], Ya[:], Yb[:], OP.add)
                    # Yfin (in place on Ya) ; gate ; ssq
                    nc.vector.tensor_tensor(Ya[:], Ya[:], xst[:], OP.add)
                    g = gpool.tile([128, 512], bf16, tag="g")
                    nc.vector.tensor_tensor(g[:], Ya[:], zsil[:, c * CH:(c + 1) * CH], OP.mult)
                    nc.scalar.activation(Ya[:], g[:], AF.Square,
                                         accum_out=ssq_sb[:, d * 16 + c: d * 16 + c + 1])
                    g_keep[(d, c)] = g

                    grp = c // 4
                    if all((d, 4 * grp + j) in g_keep for j in range(4)):
                        for ct in range(4):
                            psGT = psA.tile([128, 512], bf16, tag="psAb", bufs=2)
                            for j in range(4):
                                nc.tensor.transpose(
                                    psGT[:, 128 * j: 128 * (j + 1)],
                                    g_keep[(d, 4 * grp + j)][:, 128 * ct: 128 * (ct + 1)],
                                    idbf[:],
                                )
                            dstgt = gt_bufs[d][:, (4 * grp + ct) * 512: (4 * grp + ct + 1) * 512]
                            if (ct % 2) == 0:
                                nc.vector.tensor_copy(dstgt, psGT[:])
                            else:
                                nc.scalar.copy(dstgt, psGT[:])
                        emit_outproj(d, grp)

            nc.sync.dma_start(ssq_o[:], ssq_sb[:])

    nc.compile()
    return nc


# ---------------------------------------------------------------------------
# host side
# ---------------------------------------------------------------------------

def host_prep(inputs):
    """Build the 8 per-core input dicts (pure slicing / layout / dtype prep)."""
    x = np.ascontiguousarray(np.asarray(inputs["x"], dtype=np.float32))
    W_in = np.asarray(inputs["W_in"], dtype=np.float32)
    W_out = np.asarray(inputs["W_out"], dtype=np.float32)

    ident = np.eye(128, dtype=np.float32)
    # Gm stat layout is (s, t): forward keeps s <= t, reverse keeps s >= t,
    # block-diagonal per 32-quarter.
    maskf = np.zeros((128, 128), np.float32)
    maskr = np.zeros((128, 128), np.float32)
    for q in range(4):
        sl = slice(q * NQ, (q + 1) * NQ)
        maskf[sl, sl] = np.triu(np.ones((NQ, NQ), np.float32))
        maskr[sl, sl] = np.tril(np.ones((NQ, NQ), np.float32))
    cst_bf = np.concatenate([ident, maskf, maskr], axis=1).astype(BF16)
    cumf = np.triu(np.ones((128, 128), np.float32))    # ccum_f[t] = sum_{s<=t}
    cumr = np.tril(np.ones((128, 128), np.float32))    # ccum_r[t] = sum_{s>=t}
    cst_f = np.concatenate([ident, cumf, cumr], axis=1).astype(np.float32)
    qindm = np.zeros((4, 128), np.float32)
    for q in range(4):
        qindm[q, q * NQ:(q + 1) * NQ] = 1.0
    onesr = np.ones((1, 128), np.float32)
    zr = np.zeros((1, 128), np.float32)
    selm = np.zeros((128, 1280), np.float32)
    for d in range(2):
        base = d * 640
        if d == 0:
            for q, rr in ((1, 31), (2, 63), (3, 95)):
                selm[rr, base + q * NQ: base + (q + 1) * NQ] = 1.0
            selm[127, base + 128: base + 256] = 1.0
            for qn, rr in enumerate((31, 63, 95)):
                selm[rr, base + (2 + qn) * 128: base + (3 + qn) * 128] = 1.0
        else:
            for q, rr in ((0, 32), (1, 64), (2, 96)):
                selm[rr, base + q * NQ: base + (q + 1) * NQ] = 1.0
            selm[0, base + 128: base + 256] = 1.0
            for qn, rr in enumerate((32, 64, 96)):
                selm[rr, base + (2 + qn) * 128: base + (3 + qn) * 128] = 1.0

    per_core = []
    for core in range(8):
        b, g = divmod(core, 4)
        ch0, h0 = CH * g, HD * g
        wzc = np.ascontiguousarray(W_in[ch0:ch0 + CH].T)
        wxbcc = np.ascontiguousarray(
            np.concatenate([W_in[D_INNER + ch0: D_INNER + ch0 + CH],
                            W_in[4096:4224], W_in[4224:4352],
                            W_in[4352 + h0: 4352 + h0 + HD]], axis=0).T)
        wouts = []
        for sfx in ("_f", "_r"):
            nw = np.asarray(inputs["norm_w" + sfx], dtype=np.float32)
            weff = (W_out * nw[None, :])[:, ch0:ch0 + CH]
            wouts.append(np.ascontiguousarray(weff.T))
        woutc = np.concatenate(wouts, axis=1).astype(BF16)

        cw = np.zeros((128, 48), np.float32)
        cb = np.zeros((128, 12), np.float32)
        for d, sfx in enumerate(("_f", "_r")):
            cwf = np.asarray(inputs["conv_w" + sfx], dtype=np.float32)
            cbf = np.asarray(inputs["conv_b" + sfx], dtype=np.float32)
            rows = np.concatenate([
                cwf[ch0:ch0 + CH], cwf[D_INNER:D_INNER + 128],
                cwf[D_INNER + 128: D_INNER + 256]], axis=0)
            brows = np.concatenate([
                cbf[ch0:ch0 + CH], cbf[D_INNER:D_INNER + 128],
                cbf[D_INNER + 128: D_INNER + 256]])
            if d == 1:
                rows = rows[:, ::-1]
            for ct in range(NXBCT):
                cw[:, d * 24 + ct * 4: d * 24 + (ct + 1) * 4] = rows[ct * 128:(ct + 1) * 128]
                cb[:, d * 6 + ct] = brows[ct * 128:(ct + 1) * 128]

        dtb = np.stack([
            np.asarray(inputs["dt_bias_f"], np.float32)[h0:h0 + HD],
            np.asarray(inputs["dt_bias_r"], np.float32)[h0:h0 + HD]], axis=1)
        ar = np.zeros((1, 256), np.float32)
        for d, sfx in enumerate(("_f", "_r")):
            A = -np.exp(np.asarray(inputs["A_log" + sfx], np.float32)[h0:h0 + HD])
            ar[0, d * 128:(d + 1) * 128] = np.tile(A, NCH)

        per_core.append({
            "xT": np.ascontiguousarray(x[b].T),
            "wz": wzc, "wxbc": wxbcc, "wout": woutc,
            "convw": cw, "convb": cb, "dtbias": dtb, "arow": ar,
            "cst_bf": cst_bf, "cst_f": cst_f, "qind": qindm,
            "onesrow": onesr, "zrow": zr, "selm": selm,
        })
    return per_core


def combine(results):
    """Host unshard: sum row-parallel partials, apply the RMS row scales."""
    out = np.zeros((2, T, D_MODEL), np.float32)
    for b in range(2):
        pf = np.zeros((T, D_MODEL), np.float32)
        pr = np.zeros((T, D_MODEL), np.float32)
        sf = np.zeros(T, np.float32)
        sr = np.zeros(T, np.float32)
        for g in range(4):
            r = results[4 * b + g]
            pf += r["outT_f"].astype(np.float32).T
            pr += r["outT_r"].astype(np.float32).T
            ss = np.asarray(r["ssq"], np.float32)       # (128, 32): [t%128, dir*16+chunk]
            sf += ss[:, 0:16].T.reshape(T)
            sr += ss[:, 16:32].T.reshape(T)
        scf = 1.0 / np.sqrt(sf / D_INNER + EPS)
        scr = 1.0 / np.sqrt(sr / D_INNER + EPS)
        out[b] = scf[:, None] * pf + scr[:, None] * pr
    return out


_CACHED = {}


def kernel(**inputs):
    from concourse.bass_utils import run_bass_kernel_spmd

    assert (np.allclose(np.asarray(inputs["D_f"]), 1.0)
            and np.allclose(np.asarray(inputs["D_r"]), 1.0)), \
        "kernel assumes D skip weights == 1 (true for this problem's init)"

    if "prog" not in _CACHED:
        _CACHED["prog"] = build_program()
    nc = _CACHED["prog"]

    in_maps = host_prep(inputs)
    res = run_bass_kernel_spmd(nc, in_maps, list(range(8)))
    return combine(res.results)
